# revision 1
# baseline (speedup 1.0000x reference)
"""BinaryConv2D Trainium2 kernel.

Reference computation:
    out = conv2d(sign(x), sign(w), SAME, stride 1)   # sign(v) = +1 if v>=0 else -1
    x: (64, 56, 56, 128) f32, w: (3, 3, 128, 256) f32 -> out (64, 56, 56, 256) f32

Strategy (data-parallel over batch, 8 images per NeuronCore):
  1. SWDGE cast-DMA x f32 -> bf16 (HBM->HBM), 2 images per DMA.  The cast
     preserves sign, and only the sign bit is consumed downstream.
  2. Per image pair: HW xbar DMA-transpose (DRAM->SBUF) [6272 px, 128 ch] ->
     [128 ch, 6272 px] bf16.  Weights are binarized host-side and loaded with
     another xbar transpose.
  3. One DVE tensor_scalar op per image binarizes via bit ops on the bf16
     pattern ((v & 0x8000) | 0x3F80 -> exactly +-1.0) while scattering rows
     into a zero-padded 58x58 layout (SAME padding becomes pointer shifts).
  4. 3x3 conv = 9 accumulating matmuls per output tile.  Output stays
     pixel-major: out[px, co] = sum_taps xpad[ci, px+s].T @ w_tap[ci, co]
     with lhsT (stationary) = x tile [128ci x 116px] (2 padded rows), rhs =
     w tap [128ci x 256co], PSUM f32 [116 x 256].  All values are +-1 in
     bf16, accumulation is f32 -> arithmetic is exact.
  5. DVE copies PSUM -> SBUF stage; two large DMAs per half-image write the
     NHWC output (even rows / odd rows) back to HBM.

Built on bacc.Bacc (not raw Bass) so multi-semaphore waits are legalized
into EventSemaphore chains (TRN2 instructions hold at most one sync wait).
"""

import sys

if "/opt/trn_rl_repo" not in sys.path:
    sys.path.insert(0, "/opt/trn_rl_repo")

import numpy as np

import concourse.bacc as bacc
import concourse.bass as bass
import concourse.mybir as mybir
from concourse.tile import TileContext
from concourse.bass_utils import run_bass_kernel_spmd

N_CORES = 8
IMGS = 8  # images per core
H = W = 56
C = 128  # input channels (= contraction dim = SBUF partitions)
O = 256  # output channels
PW = 58  # padded row width
PH = 58  # padded rows per image (rows 0 and 57 are the SAME-padding rows)
PPI = PH * PW  # padded pixels per image (3364)
GUARD_L = 1  # zero guard before image 0 (tap offset -59 at tile 0)
GUARD_R = 4
TILES = H // 2  # 28 output tiles per image, 2 output rows each
F32 = mybir.dt.float32
BF16 = mybir.dt.bfloat16
U16 = mybir.dt.uint16

# tap order k = 3*di + dj ; shift in padded flat coords
TAP_SHIFTS = [PW * (di - 1) + (dj - 1) for di in range(3) for dj in range(3)]


def build_nc() -> bass.Bass:
    nc = bacc.Bacc()
    x_t = nc.dram_tensor("x", [IMGS, H, W, C], F32, kind="ExternalInput")
    # host-binarized weights, laid out [tap*co, ci] so one xbar DMA-transpose
    # loads them as [ci, tap*co]
    wbt_t = nc.dram_tensor("wbt", [9 * O, C], BF16, kind="ExternalInput")
    y_t = nc.dram_tensor("out", [IMGS, H, W, O], F32, kind="ExternalOutput")
    # per-pair bf16 bounce tensors keep DRAM dependency tracking precise
    xb_ts = [
        nc.dram_tensor(f"xb{p}", [2 * H * W, C], BF16) for p in range(IMGS // 2)
    ]

    with TileContext(nc) as tc:
        with (
            tc.tile_pool(name="const", bufs=1) as constp,
            tc.tile_pool(name="xtr", bufs=IMGS // 2) as xtrp,
            tc.tile_pool(name="stage", bufs=3) as stagep,
            tc.tile_pool(name="psum", bufs=6, space="PSUM") as psump,
        ):
            # ---- weights: single xbar transpose load of host-binarized w ----
            wb = constp.tile([C, 9 * O], BF16)
            nc.sync.dma_start(out=wb[:], in_=wbt_t[:], transpose=True)

            # ---- per-image zero-padded, channel-major input planes ----
            # Zero only the padding ranges (disjoint from the binarize write
            # range) to keep the dependency structure lean.
            xpads = []
            for i in range(IMGS):
                xp = constp.tile([C, GUARD_L + PPI + GUARD_R], BF16, tag=f"xpad{i}")
                # head: guard + top pad row + col0 of data row 1 -> [0, 60)
                nc.vector.memset(xp[:, 0:60], 0.0)
                # interior: col57 of row r + col0 of row r+1 -> [58k, 58k+2)
                nc.vector.memset(
                    xp[:, 116 : 116 + 55 * PW].rearrange("c (r w) -> c r w", w=PW)[
                        :, :, 0:2
                    ],
                    0.0,
                )
                # tail: col57 of row 56 + bottom pad row + guard
                nc.vector.memset(xp[:, 3306 : GUARD_L + PPI + GUARD_R], 0.0)
                xpads.append(xp)

            # ---- input pipeline: cast pairs, transpose pairs ----
            xtrs = {}
            for p in range(IMGS // 2):
                nc.gpsimd.dma_start(
                    out=xb_ts[p][:],
                    in_=x_t[2 * p : 2 * p + 2].rearrange("n h w c -> (n h w) c"),
                )
                xtr = xtrp.tile([C, 2 * H * W], BF16)
                nc.sync.dma_start(out=xtr[:], in_=xb_ts[p][:], transpose=True)
                xtrs[p] = xtr

            for i in range(IMGS):
                xtr = xtrs[i // 2]
                xoff = (i % 2) * H * W
                # binarize + scatter into padded rows (56 rows, stride 58)
                s0 = GUARD_L + PW + 1
                dst = xpads[i][:, s0 : s0 + H * PW].rearrange(
                    "c (r w) -> c r w", w=PW
                )[:, :, 0:W]
                src = xtr[:, xoff : xoff + H * W].rearrange("c (r w) -> c r w", w=W)
                nc.vector.tensor_scalar(
                    dst.bitcast(U16),
                    src.bitcast(U16),
                    0x8000,
                    0x3F80,
                    op0=mybir.AluOpType.bitwise_and,
                    op1=mybir.AluOpType.bitwise_or,
                )

                # ---- 28 output tiles (2 rows each) of 9 accumulating matmuls,
                # staged in half-image chunks of 14 tiles to bound SBUF ----
                HT = TILES // 2  # 14
                for half in range(2):
                    stage = stagep.tile([128, HT * O], F32)
                    st3 = stage[:].rearrange("p (t o) -> p t o", o=O)
                    for th in range(HT):
                        t = half * HT + th
                        ps = psump.tile([128, O], F32)
                        p0 = GUARD_L + PW * (1 + 2 * t)  # padded start of tile
                        for k, s in enumerate(TAP_SHIFTS):
                            a = p0 + s
                            nc.tensor.matmul(
                                ps[:116, :],
                                xpads[i][:, a : a + 116],
                                wb[:, k * O : (k + 1) * O],
                                start=(k == 0),
                                stop=(k == 8),
                            )
                        nc.vector.tensor_copy(
                            stage[:116, th * O : (th + 1) * O], ps[:116, :]
                        )

                    # ---- write out: partitions 1..56 = even rows, 59..114 odd
                    rows = y_t[i][half * 2 * HT : (half + 1) * 2 * HT]
                    ye = rows.rearrange("(r2 two) w c -> two w r2 c", two=2)
                    nc.gpsimd.dma_start(out=ye[0], in_=st3[1 : 1 + W])
                    nc.gpsimd.dma_start(out=ye[1], in_=st3[59 : 59 + W])

    nc.finalize()
    return nc


_NC_CACHE = None


def _get_nc():
    global _NC_CACHE
    if _NC_CACHE is None:
        _NC_CACHE = build_nc()
    return _NC_CACHE


def prep_wbt(w: np.ndarray) -> np.ndarray:
    """Binarize + transpose weights on host: (3,3,128,256) f32 ->
    [9*256, 128] bf16 with exact +-1 values (replicated to every core)."""
    import ml_dtypes

    wb = np.where(w >= 0, np.float32(1.0), np.float32(-1.0))
    # [di, dj, ci, co] -> [(di dj) co, ci]
    wbt = wb.transpose(0, 1, 3, 2).reshape(9 * O, C)
    return np.ascontiguousarray(wbt.astype(ml_dtypes.bfloat16))


def _ntff_hook():
    """NTFF capture context manager via the axon PJRT .so (the installed
    antenv lacks axon_hooks, so build the ctypes hook directly)."""
    sys.path.insert(0, "/root/.axon_site")
    from trn_agent_boot.trn_boot import _ntff_profile_via_ctypes

    return _ntff_profile_via_ctypes("/opt/axon/libaxon_pjrt.so")


def run(inputs: dict, profile_dir: str | None = None):
    """Run on all 8 NeuronCores. Returns (full_output, BassKernelResults)."""
    x = np.ascontiguousarray(np.asarray(inputs["x"], dtype=np.float32))
    w = np.ascontiguousarray(np.asarray(inputs["w"], dtype=np.float32))
    assert x.shape == (N_CORES * IMGS, H, W, C), x.shape
    assert w.shape == (3, 3, C, O), w.shape

    nc = _get_nc()
    wbt = prep_wbt(w)
    in_maps = [
        {"x": x[i * IMGS : (i + 1) * IMGS], "wbt": wbt} for i in range(N_CORES)
    ]
    if profile_dir is not None:
        hook = _ntff_hook()
        with hook(profile_dir, [0]):
            res = run_bass_kernel_spmd(nc, in_maps, list(range(N_CORES)))
    else:
        res = run_bass_kernel_spmd(nc, in_maps, list(range(N_CORES)))
    out = np.concatenate([res.results[i]["out"] for i in range(N_CORES)], axis=0)
    return out, res


def kernel(**inputs: np.ndarray) -> np.ndarray:
    out, _ = run(inputs)
    return out



# revision 3
# speedup vs baseline: 1.1685x; 1.1685x over previous
"""BinaryConv2D Trainium2 kernel.

Reference computation:
    out = conv2d(sign(x), sign(w), SAME, stride 1)   # sign(v) = +1 if v>=0 else -1
    x: (64, 56, 56, 128) f32, w: (3, 3, 128, 256) f32 -> out (64, 56, 56, 256) f32

Strategy (data-parallel over batch, 8 images per NeuronCore):
  1. SWDGE cast-DMA x f32 -> bf16 (HBM->HBM), 2 images per DMA.  The cast
     preserves sign, and only the sign bit is consumed downstream.
  2. Per image pair: HW xbar DMA-transpose (DRAM->SBUF) [6272 px, 128 ch] ->
     [128 ch, 6272 px] bf16.  Weights are binarized host-side and loaded with
     another xbar transpose.
  3. One DVE tensor_scalar op per image binarizes via bit ops on the bf16
     pattern ((v & 0x8000) | 0x3F80 -> exactly +-1.0) while scattering rows
     into a zero-padded 58x58 layout (SAME padding becomes pointer shifts).
  4. 3x3 conv = 9 accumulating matmuls per output tile.  Output stays
     pixel-major: out[px, co] = sum_taps xpad[ci, px+s].T @ w_tap[ci, co]
     with lhsT (stationary) = x tile [128ci x 116px] (2 padded rows), rhs =
     w tap [128ci x 256co], PSUM f32 [116 x 256].  All values are +-1 in
     bf16, accumulation is f32 -> arithmetic is exact.
  5. DVE copies PSUM -> SBUF stage; two large DMAs per half-image write the
     NHWC output (even rows / odd rows) back to HBM.

Built on bacc.Bacc (not raw Bass) so multi-semaphore waits are legalized
into EventSemaphore chains (TRN2 instructions hold at most one sync wait).
"""

import sys

if "/opt/trn_rl_repo" not in sys.path:
    sys.path.insert(0, "/opt/trn_rl_repo")

import numpy as np

import concourse.bacc as bacc
import concourse.bass as bass
import concourse.mybir as mybir
from concourse.tile import TileContext
from concourse.bass_utils import run_bass_kernel_spmd

N_CORES = 8
IMGS = 8  # images per core
H = W = 56
C = 128  # input channels (= contraction dim = SBUF partitions)
O = 256  # output channels
PW = 58  # padded row width
PH = 58  # padded rows per image (rows 0 and 57 are the SAME-padding rows)
PPI = PH * PW  # padded pixels per image (3364)
GUARD_L = 1  # zero guard before image 0 (tap offset -59 at tile 0)
GUARD_R = 16  # covers tap offset +59 with 128-col stationary at the last tile
TILES = H // 2  # 28 output tiles per image, 2 output rows each
F32 = mybir.dt.float32
BF16 = mybir.dt.bfloat16
U16 = mybir.dt.uint16

# tap order k = 3*di + dj ; shift in padded flat coords
TAP_SHIFTS = [PW * (di - 1) + (dj - 1) for di in range(3) for dj in range(3)]


def build_nc() -> bass.Bass:
    nc = bacc.Bacc()
    x_t = nc.dram_tensor("x", [IMGS, H, W, C], F32, kind="ExternalInput")
    # host-binarized weights, laid out [tap*co, ci] so one xbar DMA-transpose
    # loads them as [ci, tap*co]
    wbt_t = nc.dram_tensor("wbt", [9 * O, C], BF16, kind="ExternalInput")
    y_t = nc.dram_tensor("out", [IMGS, H, W, O], F32, kind="ExternalOutput")
    # per-pair bf16 bounce tensors keep DRAM dependency tracking precise
    xb_ts = [
        nc.dram_tensor(f"xb{p}", [2 * H * W, C], BF16) for p in range(IMGS // 2)
    ]

    with TileContext(nc) as tc:
        with (
            tc.tile_pool(name="const", bufs=1) as constp,
            tc.tile_pool(name="xtr", bufs=IMGS // 2) as xtrp,
            tc.tile_pool(name="stage", bufs=3) as stagep,
            tc.tile_pool(name="psum", bufs=6, space="PSUM") as psump,
        ):
            # ---- weights: single xbar transpose load of host-binarized w ----
            wb = constp.tile([C, 9 * O], BF16)
            nc.sync.dma_start(out=wb[:], in_=wbt_t[:], transpose=True)

            # ---- per-image zero-padded, channel-major input planes ----
            # Zero only the padding ranges (disjoint from the binarize write
            # range) to keep the dependency structure lean.
            xpads = []
            for i in range(IMGS):
                xp = constp.tile([C, GUARD_L + PPI + GUARD_R], BF16, tag=f"xpad{i}")
                # head: guard + top pad row + col0 of data row 1 -> [0, 60)
                nc.vector.memset(xp[:, 0:60], 0.0)
                # interior: col57 of row r + col0 of row r+1 -> [58k, 58k+2)
                nc.vector.memset(
                    xp[:, 116 : 116 + 55 * PW].rearrange("c (r w) -> c r w", w=PW)[
                        :, :, 0:2
                    ],
                    0.0,
                )
                # tail: col57 of row 56 + bottom pad row + guard
                nc.vector.memset(xp[:, 3306 : GUARD_L + PPI + GUARD_R], 0.0)
                xpads.append(xp)

            # ---- input pipeline: cast pairs, transpose pairs ----
            xtrs = {}
            for p in range(IMGS // 2):
                nc.gpsimd.dma_start(
                    out=xb_ts[p][:],
                    in_=x_t[2 * p : 2 * p + 2].rearrange("n h w c -> (n h w) c"),
                )
                xtr = xtrp.tile([C, 2 * H * W], BF16)
                nc.sync.dma_start(out=xtr[:], in_=xb_ts[p][:], transpose=True)
                xtrs[p] = xtr

            for i in range(IMGS):
                xtr = xtrs[i // 2]
                xoff = (i % 2) * H * W
                # binarize + scatter into padded rows (56 rows, stride 58)
                s0 = GUARD_L + PW + 1
                dst = xpads[i][:, s0 : s0 + H * PW].rearrange(
                    "c (r w) -> c r w", w=PW
                )[:, :, 0:W]
                src = xtr[:, xoff : xoff + H * W].rearrange("c (r w) -> c r w", w=W)
                nc.vector.tensor_scalar(
                    dst.bitcast(U16),
                    src.bitcast(U16),
                    0x8000,
                    0x3F80,
                    op0=mybir.AluOpType.bitwise_and,
                    op1=mybir.AluOpType.bitwise_or,
                )

                # ---- 28 output tiles (2 rows each) of 9 accumulating matmuls,
                # staged in half-image chunks of 14 tiles to bound SBUF ----
                HT = TILES // 2  # 14
                for half in range(2):
                    stage = stagep.tile([128, HT * O], F32)
                    st3 = stage[:].rearrange("p (t o) -> p t o", o=O)
                    for th in range(HT):
                        t = half * HT + th
                        ps = psump.tile([128, O], F32)
                        p0 = GUARD_L + PW * (1 + 2 * t)  # padded start of tile
                        # 128-col stationary (vs the 116 needed) so the
                        # compiler enables Fast Weight Load (NumWeights==128);
                        # partitions 116..127 of the PSUM tile are junk.
                        for k, s in enumerate(TAP_SHIFTS):
                            a = p0 + s
                            nc.tensor.matmul(
                                ps[:, :],
                                xpads[i][:, a : a + 128],
                                wb[:, k * O : (k + 1) * O],
                                start=(k == 0),
                                stop=(k == 8),
                            )
                        nc.vector.tensor_copy(
                            stage[:116, th * O : (th + 1) * O], ps[:116, :]
                        )

                    # ---- write out: partitions 1..56 = even rows, 59..114 odd
                    rows = y_t[i][half * 2 * HT : (half + 1) * 2 * HT]
                    ye = rows.rearrange("(r2 two) w c -> two w r2 c", two=2)
                    nc.gpsimd.dma_start(out=ye[0], in_=st3[1 : 1 + W])
                    nc.gpsimd.dma_start(out=ye[1], in_=st3[59 : 59 + W])

    nc.finalize()
    return nc


_NC_CACHE = None


def _get_nc():
    global _NC_CACHE
    if _NC_CACHE is None:
        _NC_CACHE = build_nc()
    return _NC_CACHE


def prep_wbt(w: np.ndarray) -> np.ndarray:
    """Binarize + transpose weights on host: (3,3,128,256) f32 ->
    [9*256, 128] bf16 with exact +-1 values (replicated to every core)."""
    import ml_dtypes

    wb = np.where(w >= 0, np.float32(1.0), np.float32(-1.0))
    # [di, dj, ci, co] -> [(di dj) co, ci]
    wbt = wb.transpose(0, 1, 3, 2).reshape(9 * O, C)
    return np.ascontiguousarray(wbt.astype(ml_dtypes.bfloat16))


def _ntff_hook():
    """NTFF capture context manager via the axon PJRT .so (the installed
    antenv lacks axon_hooks, so build the ctypes hook directly)."""
    sys.path.insert(0, "/root/.axon_site")
    from trn_agent_boot.trn_boot import _ntff_profile_via_ctypes

    return _ntff_profile_via_ctypes("/opt/axon/libaxon_pjrt.so")


def run(inputs: dict, profile_dir: str | None = None):
    """Run on all 8 NeuronCores. Returns (full_output, BassKernelResults)."""
    x = np.ascontiguousarray(np.asarray(inputs["x"], dtype=np.float32))
    w = np.ascontiguousarray(np.asarray(inputs["w"], dtype=np.float32))
    assert x.shape == (N_CORES * IMGS, H, W, C), x.shape
    assert w.shape == (3, 3, C, O), w.shape

    nc = _get_nc()
    wbt = prep_wbt(w)
    in_maps = [
        {"x": x[i * IMGS : (i + 1) * IMGS], "wbt": wbt} for i in range(N_CORES)
    ]
    if profile_dir is not None:
        hook = _ntff_hook()
        with hook(profile_dir, [0]):
            res = run_bass_kernel_spmd(nc, in_maps, list(range(N_CORES)))
    else:
        res = run_bass_kernel_spmd(nc, in_maps, list(range(N_CORES)))
    out = np.concatenate([res.results[i]["out"] for i in range(N_CORES)], axis=0)
    return out, res


def kernel(**inputs: np.ndarray) -> np.ndarray:
    out, _ = run(inputs)
    return out



# revision 6
# speedup vs baseline: 1.6086x; 1.3766x over previous
"""BinaryConv2D Trainium2 kernel (v2: fp8 DoubleRow, weights-stationary).

Reference computation:
    out = conv2d(sign(x), sign(w), SAME, stride 1)   # sign(v) = +1 if v>=0 else -1
    x: (64, 56, 56, 128) f32, w: (3, 3, 128, 256) f32 -> out (64, 56, 56, 256) f32

Strategy (data-parallel over batch, 8 images per NeuronCore):
  1. SWDGE cast-DMA x f32 -> bf16 (HBM->HBM) per image, then HW xbar
     DMA-transpose (DRAM->SBUF) to channel-major [128ci, 3136px] bf16.
  2. ACT Sign activation binarizes bf16 -> fp8e4 (+-1.0 exactly) while
     scattering rows into a zero-padded plane of 58 rows x 64 cols per
     image (SAME padding and tap shifts become pointer arithmetic).
  3. Weights are binarized host-side into fp8 tap-PAIR blocks
     [ci, 2, co_half].  The 9 taps = 4 DoubleRow pairs + 1 single.
     DoubleRow contracts 256 rows/matmul (2 taps x 128 ci): the moving
     operand is a custom 3D access pattern [ci, 2, 512] over the padded
     plane whose pair-dim stride is the tap-shift difference.  All
     values are +-1 so f32 PSUM accumulation is exact.
  4. Output tile = PSUM [128 co_half, 512 px] (8 image rows x 64).  DVE
     copies PSUM -> SBUF fp16 (exact: |out| <= 1152 < 2048), stripping
     the pad columns.  One HWDGE DMA per (window, co_half) writes the
     co-major output tensor [256, 8*3136] fp16.
  5. Host transposes [256, n, 3136] -> NHWC f32 (cheap numpy pass).

Built on bacc.Bacc so multi-semaphore waits are legalized into
EventSemaphore chains.
"""

import sys

if "/opt/trn_rl_repo" not in sys.path:
    sys.path.insert(0, "/opt/trn_rl_repo")

import numpy as np

import bass_rust
import concourse.bacc as bacc
import concourse.bass as bass
import concourse.mybir as mybir
from concourse.tile import TileContext
from concourse.bass_utils import run_bass_kernel_spmd

N_CORES = 8
IMGS = 8  # images per core
H = W = 56
C = 128  # input channels (= SBUF partitions)
O = 256  # output channels
PW = 58  # padded row width (cols 0 and 57 are the SAME-padding cols)
PH = 58  # padded rows (rows 0 and 57 are the SAME-padding rows)
PPI = PH * PW  # padded pixels per image (3364)
GUARD_L = 8
GUARD_R = 8
XP_LEN = GUARD_L + PPI + GUARD_R
NWIN = 7  # 8-output-row windows per image
NPX = 8 * PW  # window size (8 rows x 58 = 464 <= 512 psum-bank limit)
F32 = mybir.dt.float32
F16 = mybir.dt.float16
BF16 = mybir.dt.bfloat16
F8 = mybir.dt.float8e4
DR = mybir.MatmulPerfMode.DoubleRow


def shift(di, dj):
    return PW * (di - 1) + (dj - 1)


# 9 taps = 4 DoubleRow pairs + 1 single
PAIRS = [((0, 0), (0, 1)), ((0, 2), (1, 0)), ((1, 1), (1, 2)), ((2, 0), (2, 1))]
SINGLE = (2, 2)
PAIR_BASE = [shift(*a) for a, b in PAIRS]
PAIR_STRIDE = [shift(*b) - shift(*a) for a, b in PAIRS]
SINGLE_SHIFT = shift(*SINGLE)


def pair_ap(base_ap, pair_stride, n):
    """3D AP [128, 2, n]: [partition, pair(stride=pair_stride), col(stride 1)]."""
    ap = base_ap.copy()
    part = list(base_ap.ap[0])
    ap.ap = bass_rust.VecI64Pair([part, [pair_stride, 2], [1, n]])
    return ap


def build_nc() -> bass.Bass:
    nc = bacc.Bacc()
    x_t = nc.dram_tensor("x", [IMGS, H, W, C], F32, kind="ExternalInput")
    # host-binarized fp8 weights: 4 pairs x 2 halves x [ci, 2, 128] then
    # 2 single-tap halves [ci, 128]  -> [128, 2304]
    w_t = nc.dram_tensor("wall", [C, 9 * O], F8, kind="ExternalInput")
    y_t = nc.dram_tensor("out", [O, IMGS * H * W], F16, kind="ExternalOutput")
    xb_ts = [nc.dram_tensor(f"xb{i}", [H * W, C], BF16) for i in range(IMGS)]

    with TileContext(nc) as tc:
        with (
            tc.tile_pool(name="const", bufs=1) as constp,
            tc.tile_pool(name="xtr", bufs=2) as xtrp,
            tc.tile_pool(name="stage", bufs=4) as stagep,
            tc.tile_pool(name="psum", bufs=4, space="PSUM") as psump,
        ):
            wall = constp.tile([C, 9 * O], F8)
            nc.sync.dma_start(out=wall[:], in_=w_t[:])

            def w_pair(p, h):  # [ci, 2, 128] view of pair p, co-half h
                off = (2 * p + h) * O
                return wall[:, off : off + O].rearrange("c (j o) -> c j o", j=2)

            def w_single(h):  # [ci, 128]
                off = 8 * O + h * (O // 2)
                return wall[:, off : off + O // 2]

            # persistent zero-padded fp8 planes, one per image
            xpads = []
            for i in range(IMGS):
                xp = constp.tile([C, XP_LEN], F8, tag=f"xpad{i}")
                nc.vector.memset(xp[:], 0.0)
                xpads.append(xp)

            # ---- input pipeline: cast -> transpose -> binarize ----
            for i in range(IMGS):
                nc.gpsimd.dma_start(
                    out=xb_ts[i][:],
                    in_=x_t[i].rearrange("h w c -> (h w) c"),
                )
                xtr = xtrp.tile([C, H * W], BF16)
                nc.sync.dma_start(out=xtr[:], in_=xb_ts[i][:], transpose=True)
                # rows 1..56, cols 1..56 of the padded plane
                s0 = GUARD_L + PW + 1
                dst = xpads[i][:, s0 : s0 + H * PW].rearrange(
                    "c (r w) -> c r w", w=PW
                )[:, :, 0:W]
                src = xtr[:].rearrange("c (r w) -> c r w", w=W)
                nc.scalar.activation(dst, src, mybir.ActivationFunctionType.Sign)

            # ---- conv: 7 windows x 2 co-halves x (4 DR + 1 single) ----
            for i in range(IMGS):
                for win in range(NWIN):
                    q0 = GUARD_L + PW * (1 + 8 * win)
                    for h in range(2):
                        ps = psump.tile([128, NPX], F32)
                        for p in range(4):
                            a = q0 + PAIR_BASE[p]
                            rhs = pair_ap(
                                xpads[i][:, a : a + NPX], PAIR_STRIDE[p], NPX
                            )
                            nc.tensor.matmul(
                                ps[:],
                                w_pair(p, h),
                                rhs,
                                start=(p == 0),
                                stop=False,
                                perf_mode=DR,
                            )
                        a = q0 + SINGLE_SHIFT
                        nc.tensor.matmul(
                            ps[:],
                            w_single(h),
                            xpads[i][:, a : a + NPX],
                            start=False,
                            stop=True,
                        )
                        # strip pad cols during PSUM evacuation (f32 -> f16)
                        st = stagep.tile([128, 8 * W], F16)
                        nc.vector.tensor_copy(
                            st[:].rearrange("c (r w) -> c r w", w=W),
                            ps[:].rearrange("c (r w) -> c r w", w=PW)[:, :, 1 : 1 + W],
                        )
                        nc.sync.dma_start(
                            out=y_t[
                                h * 128 : (h + 1) * 128,
                                i * H * W + win * 8 * W : i * H * W + (win + 1) * 8 * W,
                            ],
                            in_=st[:],
                        )

    nc.finalize()
    return nc


_NC_CACHE = None


def _get_nc():
    global _NC_CACHE
    if _NC_CACHE is None:
        _NC_CACHE = build_nc()
    return _NC_CACHE


def prep_w(w: np.ndarray) -> np.ndarray:
    """Binarize + pack weights host-side: (3,3,128,256) f32 -> [128, 2304] fp8
    laid out as 4 pairs x 2 halves x [ci, 2tap, 128co] + 2 x [ci, 128co]."""
    import ml_dtypes

    wb = np.where(w >= 0, np.float32(1.0), np.float32(-1.0))  # [di,dj,ci,co]
    blocks = []
    for (diA, djA), (diB, djB) in PAIRS:
        for h in range(2):
            blk = np.stack(
                [
                    wb[diA, djA, :, h * 128 : (h + 1) * 128],
                    wb[diB, djB, :, h * 128 : (h + 1) * 128],
                ],
                axis=1,
            )  # [ci, 2, 128]
            blocks.append(blk.reshape(C, 256))
    di, dj = SINGLE
    for h in range(2):
        blocks.append(wb[di, dj, :, h * 128 : (h + 1) * 128])  # [ci, 128]
    wall = np.concatenate(blocks, axis=1)  # [128, 2304]
    assert wall.shape == (C, 9 * O)
    return np.ascontiguousarray(wall.astype(ml_dtypes.float8_e4m3))


def _ntff_hook():
    sys.path.insert(0, "/root/.axon_site")
    from trn_agent_boot.trn_boot import _ntff_profile_via_ctypes

    return _ntff_profile_via_ctypes("/opt/axon/libaxon_pjrt.so")


def run(inputs: dict, profile_dir: str | None = None):
    """Run on all 8 NeuronCores. Returns (full_output, BassKernelResults)."""
    x = np.ascontiguousarray(np.asarray(inputs["x"], dtype=np.float32))
    w = np.ascontiguousarray(np.asarray(inputs["w"], dtype=np.float32))
    assert x.shape == (N_CORES * IMGS, H, W, C), x.shape
    assert w.shape == (3, 3, C, O), w.shape

    nc = _get_nc()
    wall = prep_w(w)
    in_maps = [
        {"x": x[i * IMGS : (i + 1) * IMGS], "wall": wall} for i in range(N_CORES)
    ]
    if profile_dir is not None:
        hook = _ntff_hook()
        with hook(profile_dir, [0]):
            res = run_bass_kernel_spmd(nc, in_maps, list(range(N_CORES)))
    else:
        res = run_bass_kernel_spmd(nc, in_maps, list(range(N_CORES)))

    out = np.empty((N_CORES * IMGS, H, W, O), dtype=np.float32)
    for i in range(N_CORES):
        yc = np.asarray(res.results[i]["out"])  # [256, 8*3136] fp16
        out[i * IMGS : (i + 1) * IMGS] = (
            yc.astype(np.float32).reshape(O, IMGS, H, W).transpose(1, 2, 3, 0)
        )
    return out, res


def kernel(**inputs: np.ndarray) -> np.ndarray:
    out, _ = run(inputs)
    return out
